# revision 1
# baseline (speedup 1.0000x reference)
"""CoAtNet transformer block kernel for Trainium2 (8 NeuronCores).

Strategy:
  - Data-parallel over batch: 64 images -> 8 per core, no collectives.
  - Channel-major activation layout [C, N] on chip (x arrives as (C, H*W)).
  - All matmuls in bf16 (fp32 PSUM accumulation); LN stats / residuals fp32.
  - LayerNorm gamma/beta folded into QKV weights host-side. Q/K projections
    run on raw (un-normalized) bf16 x so they never wait on the LN stats
    chain; the per-token (mean, rstd) correction is applied to the PSUM
    result as rstd*psum + mr*colsum(w) (+bias) on the vector/scalar engines.
  - Attention computed transposed (simT[m, n] = k@q.T + biasT) so softmax
    normalization is a column sum obtained for free from a ones-column in
    the V matmul; bias applied as precomputed exp(biasT) multiplier.
  - Batches processed in pairs so the moving free dim is 392 (hides
    LDWEIGHTS under the matmul stream); per-head sim matmuls packed two
    heads at a time into disjoint PE row groups.
  - FFN: per 128-wide h1 chunk, gelu then immediately accumulate into six
    persistent output PSUM banks (no full h1 materialization).
"""

import numpy as np
import ml_dtypes

H = 14
W = 14
C = 768
HEADS = 12
EXPAND = 4
N = H * W  # 196
B = 64
NCORES = 8
BPC = B // NCORES  # 8 batches per core
DH = C // HEADS  # 64
KC = C // 128  # 6 chunks of 128 channels
F = C * EXPAND  # 3072
KF = F // 128  # 24
NPAIR = 2 * N  # 392
M0, M1 = 128, N - 128  # token chunks 128 + 68
MCHUNKS = ((0, M0), (M0, M1))


def _relative_indices():
    gy, gx = np.meshgrid(np.arange(H), np.arange(W), indexing="ij")
    py, px = gy.reshape(-1), gx.reshape(-1)
    rel_y = py[None, :] - py[:, None] + H
    rel_x = px[None, :] - px[:, None] + W
    return rel_y * W + rel_x  # (N, N) int


def _build_bass():
    import concourse.bacc as bacc
    import concourse.mybir as mybir
    import concourse.tile as tile

    f32 = mybir.dt.float32
    bf16 = mybir.dt.bfloat16
    AF = mybir.ActivationFunctionType
    OP = mybir.AluOpType

    nc = bacc.Bacc("TRN2")

    # ---- DRAM parameters (per core) ----
    x_in = nc.declare_dram_parameter("x", [BPC, C, N], f32, isOutput=False)
    wq_d = nc.declare_dram_parameter("wq", [C, C], bf16, isOutput=False)
    wk_d = nc.declare_dram_parameter("wk", [C, C], bf16, isOutput=False)
    wv_d = nc.declare_dram_parameter("wv", [C, C], bf16, isOutput=False)
    wo_d = nc.declare_dram_parameter("wo", [C, C], bf16, isOutput=False)
    w1_d = nc.declare_dram_parameter("w1", [C, F], bf16, isOutput=False)
    w2_d = nc.declare_dram_parameter("w2", [F, C], bf16, isOutput=False)
    bq_d = nc.declare_dram_parameter("bq", [C], f32, isOutput=False)
    bk_d = nc.declare_dram_parameter("bk", [C], f32, isOutput=False)
    bo_d = nc.declare_dram_parameter("bo", [C], f32, isOutput=False)
    b1_d = nc.declare_dram_parameter("b1", [F], f32, isOutput=False)
    b2_d = nc.declare_dram_parameter("b2", [C], f32, isOutput=False)
    wsq_d = nc.declare_dram_parameter("wsq", [C], f32, isOutput=False)
    wsk_d = nc.declare_dram_parameter("wsk", [C], f32, isOutput=False)
    # exp(biasT) per head, token-chunked: [128, HEADS, N] and [68, HEADS, N]
    eb0_d = nc.declare_dram_parameter("eb0", [M0, HEADS, N], bf16, isOutput=False)
    eb1_d = nc.declare_dram_parameter("eb1", [M1, HEADS, N], bf16, isOutput=False)
    out_d = nc.declare_dram_parameter("out", [BPC, C, N], f32, isOutput=True)

    def ld(pool, name, dram, shape, pat):
        t = pool.tile(shape, dram.dtype, name=name)
        nc.sync.dma_start(t[:], dram.ap().rearrange(pat, p=128) if pat else dram.ap())
        return t

    with tile.TileContext(nc) as tc:
        with (
            tc.tile_pool(name="wpool", bufs=1) as wpool,
            tc.tile_pool(name="acts", bufs=1) as acts,
            tc.tile_pool(name="xio", bufs=2) as xio,
            tc.tile_pool(name="small", bufs=2) as small,
            tc.tile_pool(name="psum", bufs=1, space="PSUM") as pp,
        ):
            # ---- resident weights; issue order = DMA priority order ----
            wq_sb = ld(wpool, "wq_sb", wq_d, [128, KC, C], "(ko p) m -> p ko m")
            wk_sb = ld(wpool, "wk_sb", wk_d, [128, KC, C], "(ko p) m -> p ko m")
            bq_sb = ld(wpool, "bq_sb", bq_d, [128, KC], "(mo p) -> p mo")
            bk_sb = ld(wpool, "bk_sb", bk_d, [128, KC], "(mo p) -> p mo")
            wsq_sb = ld(wpool, "wsq_sb", wsq_d, [128, KC], "(mo p) -> p mo")
            wsk_sb = ld(wpool, "wsk_sb", wsk_d, [128, KC], "(mo p) -> p mo")
            wv_sb = ld(wpool, "wv_sb", wv_d, [128, KC, C], "(ko p) m -> p ko m")
            eb0_sb = ld(wpool, "eb0_sb", eb0_d, [M0, HEADS, N], None)
            eb1_sb = ld(wpool, "eb1_sb", eb1_d, [M1, HEADS, N], None)
            bo_sb = ld(wpool, "bo_sb", bo_d, [128, KC], "(mo p) -> p mo")
            b1_sb = ld(wpool, "b1_sb", b1_d, [128, KF], "(mo p) -> p mo")
            b2_sb = ld(wpool, "b2_sb", b2_d, [128, KC], "(mo p) -> p mo")
            wo_sb = ld(wpool, "wo_sb", wo_d, [128, KC, C], "(ko p) m -> p ko m")
            w1_sb = ld(wpool, "w1_sb", w1_d, [128, KC, F], "(ko p) m -> p ko m")
            w2_sb = ld(wpool, "w2_sb", w2_d, [128, KF, C], "(ko p) m -> p ko m")
            ones_sb = wpool.tile([128, 1], bf16, name="ones_sb")
            nc.vector.memset(ones_sb[:], 1.0)
            eps_sb = wpool.tile([1, 1], f32, name="eps_sb")
            nc.vector.memset(eps_sb[:], 1e-5)
            zero_sb = wpool.tile([128, 1], f32, name="zero_sb")
            nc.vector.memset(zero_sb[:], 0.0)

            eb_sb = (eb0_sb, eb1_sb)

            def load_x(pair):
                b0 = 2 * pair
                t = xio.tile([128, 2 * KC, N], f32, name="xf", tag="xf")
                nc.sync.dma_start(
                    t[:],
                    x_in.ap()[b0 : b0 + 2].rearrange(
                        "b (ko p) n -> p (b ko) n", p=128
                    ),
                )
                return t

            next_xf = load_x(0)
            for pair in range(BPC // 2):
                b0 = 2 * pair
                xf = next_xf
                if pair + 1 < BPC // 2:
                    next_xf = load_x(pair + 1)
                # pair view: [128, KC, 2, N] (chunk-major, batch inner)
                xfv = xf.rearrange("p (b k) n -> p k b n", b=2)

                # ---- bf16 cast of x (DVE) + LN stats matmuls ----
                xbf = acts.tile([128, KC, 2, N], bf16, name="xbf", tag="xbf")
                s_ps = pp.tile([1, NPAIR], f32, name="s_ps", tag="x0")
                q_ps = pp.tile([1, NPAIR], f32, name="q_ps", tag="x1")
                for k in range(KC):
                    nc.vector.tensor_copy(xbf[:, k], xfv[:, k])
                    xsq = small.tile([128, 2, N], bf16, name="xsq", tag="xsq")
                    nc.gpsimd.tensor_mul(xsq[:], xbf[:, k], xbf[:, k])
                    nc.tensor.matmul(
                        s_ps[:], ones_sb[:], xbf[:, k],
                        start=(k == 0), stop=(k == KC - 1),
                    )
                    nc.tensor.matmul(
                        q_ps[:], ones_sb[:], xsq[:],
                        start=(k == 0), stop=(k == KC - 1),
                    )

                # ---- LN stats chain (off PE critical path) ----
                mu = small.tile([1, NPAIR], f32, name="mu", tag="mu", bufs=1)
                nc.vector.tensor_scalar_mul(mu[:], s_ps[:], 1.0 / C)
                var = small.tile([1, NPAIR], f32, name="var", tag="var", bufs=1)
                # var = (mu * -mu) + sq/C
                nc.vector.scalar_tensor_tensor(
                    var[:], mu[:], -1.0, mu[:], OP.mult, OP.mult
                )
                nc.vector.scalar_tensor_tensor(
                    var[:], q_ps[:], 1.0 / C, var[:], OP.mult, OP.add
                )
                std = small.tile([1, NPAIR], f32, name="std", tag="std", bufs=1)
                nc.scalar.activation(
                    std[:], var[:], AF.Sqrt, bias=eps_sb[:], scale=1.0
                )
                rstd = small.tile([1, NPAIR], f32, name="rstd", tag="rstd", bufs=1)
                nc.vector.reciprocal(rstd[:], std[:])
                mr = small.tile([1, NPAIR], f32, name="mr", tag="mr", bufs=1)
                nc.vector.scalar_tensor_tensor(
                    mr[:], mu[:], -1.0, rstd[:], OP.mult, OP.mult
                )
                rstd_b = small.tile(
                    [128, NPAIR], f32, name="rstd_b", tag="rstd_b", bufs=1
                )
                nc.gpsimd.partition_broadcast(rstd_b[:], rstd[:])
                mr_b = small.tile([128, NPAIR], f32, name="mr_b", tag="mr_b", bufs=1)
                nc.gpsimd.partition_broadcast(mr_b[:], mr[:])
                rstd_bv = rstd_b.rearrange("p (b n) -> p b n", b=2)
                mr_bv = mr_b.rearrange("p (b n) -> p b n", b=2)

                # ---- Q/K projections from RAW x_bf; LN applied post-hoc:
                #      qT = rstd*(w'.T@x) + mr*colsum(w') + b' ----
                qT = acts.tile([128, KC, NPAIR], bf16, name="qT", tag="qT")
                kT = acts.tile([128, KC, NPAIR], bf16, name="kT", tag="kT")
                for dst, w_sb, ws_sb, bias_sb in (
                    (qT, wq_sb, wsq_sb, bq_sb),
                    (kT, wk_sb, wsk_sb, bk_sb),
                ):
                    for m in range(KC):
                        ps = pp.tile(
                            [128, NPAIR], f32, name="ps_qk", tag="mm", bufs=2
                        )
                        for k in range(KC):
                            nc.tensor.matmul(
                                ps[:],
                                w_sb[:, k, 128 * m : 128 * (m + 1)],
                                xbf[:, k],
                                start=(k == 0),
                                stop=(k == KC - 1),
                            )
                        t1 = small.tile([128, NPAIR], f32, name="t1", tag="t1")
                        nc.vector.tensor_mul(t1[:], ps[:], rstd_b[:])
                        t2 = small.tile([128, NPAIR], bf16, name="t2", tag="t2")
                        nc.vector.scalar_tensor_tensor(
                            t2[:], mr_b[:], ws_sb[:, m : m + 1], t1[:],
                            OP.mult, OP.add,
                        )
                        nc.scalar.activation(
                            dst[:, m, :], t2[:], AF.Identity,
                            bias=bias_sb[:, m : m + 1], scale=1.0,
                        )

                # ---- xn = x*rstd + mr (bf16; only feeds the V projection) ----
                xn = acts.tile([128, KC, NPAIR], bf16, name="xn", tag="xn")
                xnv = xn.rearrange("p k (b n) -> p k b n", b=2)
                for k in range(KC):
                    t = small.tile([128, NPAIR], f32, name="t_ln", tag="t1")
                    nc.vector.tensor_mul(
                        t.rearrange("p (b n) -> p b n", b=2), xfv[:, k], rstd_bv
                    )
                    nc.vector.tensor_add(
                        xnv[:, k], t.rearrange("p (b n) -> p b n", b=2), mr_bv
                    )

                # ---- V projection (token-major, per batch, 65-strided heads) ----
                v_sb = []
                for j in range(2):
                    v_c = []
                    for ci, (mstart, mlen) in enumerate(MCHUNKS):
                        vt = acts.tile(
                            [128, HEADS, DH + 1], bf16, name=f"v_{j}_{ci}",
                            tag=f"v_{j}_{ci}",
                        )
                        for s in range(2):  # halves of c_out (6 heads each)
                            pv = pp.tile(
                                [128, 384], f32, name="pv", tag="mm", bufs=2
                            )
                            for k in range(KC):
                                nc.tensor.matmul(
                                    pv[:mlen],
                                    xn[:, k, j * N + mstart : j * N + mstart + mlen],
                                    wv_sb[:, k, 384 * s : 384 * (s + 1)],
                                    start=(k == 0),
                                    stop=(k == KC - 1),
                                )
                            nc.scalar.activation(
                                vt[:mlen, 6 * s : 6 * (s + 1), 0:DH],
                                pv[:mlen].rearrange("p (h d) -> p h d", h=6),
                                AF.Identity, bias=zero_sb[:mlen], scale=1.0,
                            )
                        nc.vector.memset(vt[:mlen, :, DH : DH + 1], 1.0)
                        v_c.append(vt)
                    v_sb.append(v_c)

                # ---- attention: head-pairs packed into PE row groups;
                #      emission software-pipelined one pair ahead ----
                OT = acts.tile([128, KC, NPAIR], bf16, name="OT", tag="OT")

                def emit_sims(j, hp):
                    cb = j * N
                    sims, ets = [], []
                    for hh in range(2):  # heads 2*hp, 2*hp+1
                        prow = 64 * hh
                        sim = pp.tile(
                            [128, 2, N], f32, name="sim", tag=f"st{hh}"
                        )
                        for ci, (mstart, mlen) in enumerate(MCHUNKS):
                            nc.tensor.matmul(
                                sim[:mlen, ci],
                                kT[prow : prow + 64, hp,
                                   cb + mstart : cb + mstart + mlen],
                                qT[prow : prow + 64, hp, cb : cb + N],
                                start=True, stop=True,
                            )
                        et = small.tile(
                            [128, 2, N], bf16, name="et", tag=f"et{hh}"
                        )
                        nc.scalar.activation(
                            et[:], sim[:], AF.Exp, bias=0.0, scale=1.0
                        )
                        for ci, (mstart, mlen) in enumerate(MCHUNKS):
                            nc.vector.tensor_mul(
                                et[:mlen, ci], et[:mlen, ci],
                                eb_sb[ci][:mlen, 2 * hp + hh, :],
                            )
                        sims.append(sim)
                        ets.append(et)
                    return ets

                def emit_omms(j, hp, ets):
                    cb = j * N
                    for hh in range(2):
                        h = 2 * hp + hh
                        _otags = ["at0", "at1", "x0", "x1"]
                        ops = pp.tile(
                            [128, N], f32, name="ops", tag=_otags[(2 * hp + hh) % 4]
                        )
                        for ci, (mstart, mlen) in enumerate(MCHUNKS):
                            nc.tensor.matmul(
                                ops[: DH + 1],
                                v_sb[j][ci][:mlen, h, :],
                                ets[hh][:mlen, ci],
                                start=(ci == 0), stop=(ci == 1),
                            )
                        rec = small.tile([1, N], f32, name="rec", tag="rec")
                        nc.vector.reciprocal(rec[:], ops[DH : DH + 1, :])
                        rec_b = small.tile([64, N], f32, name="rec_b", tag="rec_b")
                        nc.gpsimd.partition_broadcast(rec_b[:], rec[:])
                        nc.vector.tensor_mul(
                            OT[64 * hh : 64 * hh + 64, hp, cb : cb + N],
                            ops[0:DH, :], rec_b[:],
                        )

                prev = None
                for j in range(2):
                    for hp in range(HEADS // 2):
                        ets = emit_sims(j, hp)
                        if prev is not None:
                            emit_omms(*prev)
                        prev = (j, hp, ets)
                emit_omms(*prev)

                # ---- out projection + residual 1 ----
                y32 = acts.tile([128, KC, NPAIR], f32, name="y32", tag="y32")
                ybf = acts.tile([128, KC, NPAIR], bf16, name="ybf", tag="ybf")
                for m in range(KC):
                    po = pp.tile([128, NPAIR], f32, name="po", tag="mm", bufs=2)
                    for k in range(KC):
                        nc.tensor.matmul(
                            po[:],
                            wo_sb[:, k, 128 * m : 128 * (m + 1)],
                            OT[:, k, :],
                            start=(k == 0),
                            stop=(k == KC - 1),
                        )
                    nc.vector.scalar_tensor_tensor(
                        y32[:, m, :].rearrange("p (b n) -> p b n", b=2),
                        po.rearrange("p (b n) -> p b n", b=2),
                        bo_sb[:, m : m + 1],
                        xfv[:, m],
                        OP.add, OP.add,
                    )
                    nc.scalar.activation(
                        ybf[:, m, :], y32[:, m, :], AF.Identity,
                        bias=zero_sb[:], scale=1.0,
                    )

                # ---- FFN fused: h1 chunk -> gelu -> accumulate into 6 psum ----
                _ptags = ["st0", "st1", "at0", "at1", "x0", "x1"]
                pouts = [
                    pp.tile([128, NPAIR], f32, name=f"pout{o}", tag=_ptags[o])
                    for o in range(KC)
                ]
                for mf in range(KF):
                    p1 = pp.tile([128, NPAIR], f32, name="p1", tag="mm", bufs=2)
                    for k in range(KC):
                        nc.tensor.matmul(
                            p1[:],
                            w1_sb[:, k, 128 * mf : 128 * (mf + 1)],
                            ybf[:, k, :],
                            start=(k == 0),
                            stop=(k == KC - 1),
                        )
                    h1c = small.tile([128, NPAIR], bf16, name="h1c", tag="h1c")
                    nc.scalar.activation(
                        h1c[:], p1[:], AF.Gelu, bias=b1_sb[:, mf : mf + 1], scale=1.0
                    )
                    for o in range(KC):
                        nc.tensor.matmul(
                            pouts[o][:],
                            w2_sb[:, mf, 128 * o : 128 * (o + 1)],
                            h1c[:],
                            start=(mf == 0),
                            stop=(mf == KF - 1),
                        )

                # ---- residual 2 + store ----
                o32 = xio.tile([128, 2 * KC, N], f32, name="o32", tag="xf")
                o32v = o32.rearrange("p (b k) n -> p k b n", b=2)
                for o in range(KC):
                    nc.vector.scalar_tensor_tensor(
                        o32v[:, o],
                        pouts[o].rearrange("p (b n) -> p b n", b=2),
                        b2_sb[:, o : o + 1],
                        y32[:, o, :].rearrange("p (b n) -> p b n", b=2),
                        OP.add, OP.add,
                    )
                nc.sync.dma_start(
                    out_d.ap()[b0 : b0 + 2].rearrange(
                        "b (ko p) n -> p (b ko) n", p=128
                    ),
                    o32[:],
                )

    nc.finalize()
    return nc


_CACHE = {}


def prepare_in_maps(inputs):
    x = np.asarray(inputs["x"], dtype=np.float32)  # (64, 768, 14, 14)
    ln_g = np.asarray(inputs["ln_g"], dtype=np.float32)
    ln_b = np.asarray(inputs["ln_b"], dtype=np.float32)
    wq = np.asarray(inputs["wq"], dtype=np.float32)
    bq = np.asarray(inputs["bq"], dtype=np.float32)
    wk = np.asarray(inputs["wk"], dtype=np.float32)
    bk = np.asarray(inputs["bk"], dtype=np.float32)
    wv = np.asarray(inputs["wv"], dtype=np.float32)
    bv = np.asarray(inputs["bv"], dtype=np.float32)
    wo = np.asarray(inputs["wo"], dtype=np.float32)
    bo = np.asarray(inputs["bo"], dtype=np.float32)
    w1 = np.asarray(inputs["w1"], dtype=np.float32)
    b1 = np.asarray(inputs["b1"], dtype=np.float32)
    w2 = np.asarray(inputs["w2"], dtype=np.float32)
    b2 = np.asarray(inputs["b2"], dtype=np.float32)
    rel_bias = np.asarray(inputs["rel_bias"], dtype=np.float32)

    bf = ml_dtypes.bfloat16

    # Fold LayerNorm gamma into QKV weights, beta into their biases.
    wqp_f = ln_g[:, None] * wq
    wkp_f = ln_g[:, None] * wk
    wvp_f = ln_g[:, None] * wv
    bqp = (ln_b @ wq + bq).astype(np.float32)
    bkp = (ln_b @ wk + bk).astype(np.float32)
    bvp = (ln_b @ wv + bv).astype(np.float32)
    # V bias commutes through softmax (rows sum to 1): fold into out-proj bias.
    bop = (bo + bvp @ wo).astype(np.float32)
    # Column sums of the folded Q/K weights for the post-hoc mean correction.
    # Use the bf16-rounded weights so the correction matches the matmul.
    wqp = wqp_f.astype(bf)
    wkp = wkp_f.astype(bf)
    wsq = wqp.astype(np.float32).sum(axis=0).astype(np.float32)
    wsk = wkp.astype(np.float32).sum(axis=0).astype(np.float32)

    # Relative position bias, transposed per head, exponentiated.
    rel_idx = _relative_indices()
    bias = rel_bias[:, rel_idx]  # (HEADS, N, N) : bias[h, n, m]
    ebT = np.exp(bias.transpose(0, 2, 1))  # (HEADS, m, n)
    eb_m = ebT.transpose(1, 0, 2)  # (m, HEADS, n)
    eb0 = np.ascontiguousarray(eb_m[:M0]).astype(bf)
    eb1 = np.ascontiguousarray(eb_m[M0:]).astype(bf)

    common = {
        "wq": wqp, "wk": wkp, "wv": wvp_f.astype(bf),
        "wo": wo.astype(bf), "w1": w1.astype(bf), "w2": w2.astype(bf),
        "bq": bqp, "bk": bkp, "bo": bop,
        "b1": b1.astype(np.float32), "b2": b2.astype(np.float32),
        "wsq": wsq, "wsk": wsk,
        "eb0": eb0, "eb1": eb1,
    }

    x_flat = x.reshape(B, C, N)
    in_maps = []
    for c in range(NCORES):
        m = dict(common)
        m["x"] = np.ascontiguousarray(x_flat[c * BPC : (c + 1) * BPC])
        in_maps.append(m)
    return in_maps


def kernel(**inputs):
    import sys

    if "/opt/trn_rl_repo" not in sys.path:
        sys.path.insert(0, "/opt/trn_rl_repo")
    from concourse.bass_utils import run_bass_kernel_spmd

    in_maps = prepare_in_maps(inputs)

    if "nc" not in _CACHE:
        _CACHE["nc"] = _build_bass()
    nc = _CACHE["nc"]

    res = run_bass_kernel_spmd(nc, in_maps, core_ids=list(range(NCORES)))
    _CACHE["last_res"] = res
    outs = [r["out"] for r in res.results]
    full = np.concatenate(outs, axis=0)  # (64, 768, 196)
    return full.reshape(B, C, H, W).astype(np.float32)



# revision 7
# speedup vs baseline: 1.0755x; 1.0755x over previous
"""CoAtNet transformer block kernel for Trainium2 (8 NeuronCores).

Strategy:
  - Data-parallel over batch: 64 images -> 8 per core, no collectives.
  - Channel-major activation layout [C, N] on chip (x arrives as (C, H*W)).
  - All matmuls in bf16 (fp32 PSUM accumulation); LN stats / residuals fp32.
  - LayerNorm gamma/beta folded into QKV weights host-side. Q/K projections
    run on raw (un-normalized) bf16 x so they never wait on the LN stats
    chain; the per-token (mean, rstd) correction is applied to the PSUM
    result as rstd*psum + mr*colsum(w) (+bias) on the vector/scalar engines.
  - Attention computed transposed (simT[m, n] = k@q.T + biasT) so softmax
    normalization is a column sum obtained for free from a ones-column in
    the V matmul; bias applied as precomputed exp(biasT) multiplier.
  - Batches processed in pairs so the moving free dim is 392 (hides
    LDWEIGHTS under the matmul stream); per-head sim matmuls packed two
    heads at a time into disjoint PE row groups.
  - FFN: per 128-wide h1 chunk, gelu then immediately accumulate into six
    persistent output PSUM banks (no full h1 materialization).
"""

import numpy as np
import ml_dtypes

H = 14
W = 14
C = 768
HEADS = 12
EXPAND = 4
N = H * W  # 196
B = 64
NCORES = 8
BPC = B // NCORES  # 8 batches per core
DH = C // HEADS  # 64
KC = C // 128  # 6 chunks of 128 channels
F = C * EXPAND  # 3072
KF = F // 128  # 24
NPAIR = 2 * N  # 392
M0, M1 = 128, N - 128  # token chunks 128 + 68
MCHUNKS = ((0, M0), (M0, M1))


def _relative_indices():
    gy, gx = np.meshgrid(np.arange(H), np.arange(W), indexing="ij")
    py, px = gy.reshape(-1), gx.reshape(-1)
    rel_y = py[None, :] - py[:, None] + H
    rel_x = px[None, :] - px[:, None] + W
    return rel_y * W + rel_x  # (N, N) int


def _build_bass():
    import concourse.bacc as bacc
    import concourse.mybir as mybir
    import concourse.tile as tile

    f32 = mybir.dt.float32
    bf16 = mybir.dt.bfloat16
    AF = mybir.ActivationFunctionType
    OP = mybir.AluOpType

    nc = bacc.Bacc("TRN2")

    # ---- DRAM parameters (per core) ----
    x_in = nc.declare_dram_parameter("x", [BPC, C, N], f32, isOutput=False)
    wq_d = nc.declare_dram_parameter("wq", [C, C], bf16, isOutput=False)
    wk_d = nc.declare_dram_parameter("wk", [C, C], bf16, isOutput=False)
    wv_d = nc.declare_dram_parameter("wv", [C, C], bf16, isOutput=False)
    wo_d = nc.declare_dram_parameter("wo", [C, C], bf16, isOutput=False)
    w1_d = nc.declare_dram_parameter("w1", [C, F], bf16, isOutput=False)
    w2_d = nc.declare_dram_parameter("w2", [F, C], bf16, isOutput=False)
    bq_d = nc.declare_dram_parameter("bq", [C], f32, isOutput=False)
    bk_d = nc.declare_dram_parameter("bk", [C], f32, isOutput=False)
    bo_d = nc.declare_dram_parameter("bo", [C], f32, isOutput=False)
    b1_d = nc.declare_dram_parameter("b1", [F], f32, isOutput=False)
    b2_d = nc.declare_dram_parameter("b2", [C], f32, isOutput=False)
    wsq_d = nc.declare_dram_parameter("wsq", [C], f32, isOutput=False)
    wsk_d = nc.declare_dram_parameter("wsk", [C], f32, isOutput=False)
    # exp(biasT) per head, token-chunked: [128, HEADS, N] and [68, HEADS, N]
    eb0_d = nc.declare_dram_parameter("eb0", [M0, HEADS, N], bf16, isOutput=False)
    eb1_d = nc.declare_dram_parameter("eb1", [M1, HEADS, N], bf16, isOutput=False)
    out_d = nc.declare_dram_parameter("out", [BPC, C, N], f32, isOutput=True)

    def ld(pool, name, dram, shape, pat):
        t = pool.tile(shape, dram.dtype, name=name)
        nc.sync.dma_start(t[:], dram.ap().rearrange(pat, p=128) if pat else dram.ap())
        return t

    with tile.TileContext(nc) as tc:
        with (
            tc.tile_pool(name="wpool", bufs=1) as wpool,
            tc.tile_pool(name="acts", bufs=1) as acts,
            tc.tile_pool(name="xio", bufs=2) as xio,
            tc.tile_pool(name="small", bufs=2) as small,
            tc.tile_pool(name="psum", bufs=1, space="PSUM") as pp,
        ):
            def load_x(pair):
                b0 = 2 * pair
                t = xio.tile([128, 2 * KC, N], f32, name="xf", tag="xf", bufs=2)
                nc.sync.dma_start(
                    t[:],
                    x_in.ap()[b0 : b0 + 2].rearrange(
                        "b (ko p) n -> p (b ko) n", p=128
                    ),
                )
                return t

            # ---- DMA issue order = arrival order: x for the first pairs
            # first, then weights in order of first use.
            next_xf = load_x(0)
            wq_sb = ld(wpool, "wq_sb", wq_d, [128, KC, C], "(ko p) m -> p ko m")
            wk_sb = ld(wpool, "wk_sb", wk_d, [128, KC, C], "(ko p) m -> p ko m")
            bq_sb = ld(wpool, "bq_sb", bq_d, [128, KC], "(mo p) -> p mo")
            bk_sb = ld(wpool, "bk_sb", bk_d, [128, KC], "(mo p) -> p mo")
            wsq_sb = ld(wpool, "wsq_sb", wsq_d, [128, KC], "(mo p) -> p mo")
            wsk_sb = ld(wpool, "wsk_sb", wsk_d, [128, KC], "(mo p) -> p mo")
            wv_sb = ld(wpool, "wv_sb", wv_d, [128, KC, C], "(ko p) m -> p ko m")
            eb0_sb = ld(wpool, "eb0_sb", eb0_d, [M0, HEADS, N], None)
            eb1_sb = ld(wpool, "eb1_sb", eb1_d, [M1, HEADS, N], None)
            bo_sb = ld(wpool, "bo_sb", bo_d, [128, KC], "(mo p) -> p mo")
            wo_sb = ld(wpool, "wo_sb", wo_d, [128, KC, C], "(ko p) m -> p ko m")
            b1_sb = ld(wpool, "b1_sb", b1_d, [128, KF], "(mo p) -> p mo")
            w1_sb = ld(wpool, "w1_sb", w1_d, [128, KC, F], "(ko p) m -> p ko m")
            b2_sb = ld(wpool, "b2_sb", b2_d, [128, KC], "(mo p) -> p mo")
            w2_sb = ld(wpool, "w2_sb", w2_d, [128, KF, C], "(ko p) m -> p ko m")
            ones_sb = wpool.tile([128, 1], bf16, name="ones_sb")
            nc.vector.memset(ones_sb[:], 1.0)
            eps_sb = wpool.tile([1, 1], f32, name="eps_sb")
            nc.vector.memset(eps_sb[:], 1e-5)
            zero_sb = wpool.tile([128, 1], f32, name="zero_sb")
            nc.vector.memset(zero_sb[:], 0.0)

            eb_sb = (eb0_sb, eb1_sb)
            for pair in range(BPC // 2):
                b0 = 2 * pair
                xf = next_xf
                if pair + 1 < BPC // 2:
                    next_xf = load_x(pair + 1)
                # pair view: [128, KC, 2, N] (chunk-major, batch inner)
                xfv = xf.rearrange("p (b k) n -> p k b n", b=2)

                # ---- bf16 cast of x (DVE) + LN stats matmuls ----
                xbf = acts.tile([128, KC, 2, N], bf16, name="xbf", tag="xbf")
                s_ps = pp.tile([1, NPAIR], f32, name="s_ps", tag="x0")
                q_ps = pp.tile([1, NPAIR], f32, name="q_ps", tag="x1")
                for k in range(KC):
                    nc.vector.tensor_copy(xbf[:, k], xfv[:, k])
                    xsq = small.tile([128, 2, N], bf16, name="xsq", tag="xsq")
                    nc.gpsimd.tensor_mul(xsq[:], xbf[:, k], xbf[:, k])
                    nc.tensor.matmul(
                        s_ps[:], ones_sb[:], xbf[:, k],
                        start=(k == 0), stop=(k == KC - 1),
                    )
                    nc.tensor.matmul(
                        q_ps[:], ones_sb[:], xsq[:],
                        start=(k == 0), stop=(k == KC - 1),
                    )

                # ---- LN stats chain (off PE critical path) ----
                mu = small.tile([1, NPAIR], f32, name="mu", tag="mu", bufs=1)
                nc.vector.tensor_scalar_mul(mu[:], s_ps[:], 1.0 / C)
                var = small.tile([1, NPAIR], f32, name="var", tag="var", bufs=1)
                # var = (mu * -mu) + sq/C
                nc.vector.scalar_tensor_tensor(
                    var[:], mu[:], -1.0, mu[:], OP.mult, OP.mult
                )
                nc.vector.scalar_tensor_tensor(
                    var[:], q_ps[:], 1.0 / C, var[:], OP.mult, OP.add
                )
                std = small.tile([1, NPAIR], f32, name="std", tag="std", bufs=1)
                nc.scalar.activation(
                    std[:], var[:], AF.Sqrt, bias=eps_sb[:], scale=1.0
                )
                rstd = small.tile([1, NPAIR], f32, name="rstd", tag="rstd", bufs=1)
                nc.vector.reciprocal(rstd[:], std[:])
                mr = small.tile([1, NPAIR], f32, name="mr", tag="mr", bufs=1)
                nc.vector.scalar_tensor_tensor(
                    mr[:], mu[:], -1.0, rstd[:], OP.mult, OP.mult
                )
                rstd_b = small.tile(
                    [128, NPAIR], f32, name="rstd_b", tag="rstd_b", bufs=1
                )
                nc.gpsimd.partition_broadcast(rstd_b[:], rstd[:])
                mr_b = small.tile([128, NPAIR], f32, name="mr_b", tag="mr_b", bufs=1)
                nc.gpsimd.partition_broadcast(mr_b[:], mr[:])
                rstd_bv = rstd_b.rearrange("p (b n) -> p b n", b=2)
                mr_bv = mr_b.rearrange("p (b n) -> p b n", b=2)

                # ---- Q/K projections from RAW x_bf; LN applied post-hoc:
                #      qT = rstd*(w'.T@x) + mr*colsum(w') + b' ----
                qT = acts.tile([128, KC, NPAIR], bf16, name="qT", tag="qT")
                kT = acts.tile([128, KC, NPAIR], bf16, name="kT", tag="kT")
                for dst, w_sb, ws_sb, bias_sb in (
                    (qT, wq_sb, wsq_sb, bq_sb),
                    (kT, wk_sb, wsk_sb, bk_sb),
                ):
                    for m in range(KC):
                        ps = pp.tile(
                            [128, NPAIR], f32, name="ps_qk", tag="mm", bufs=2
                        )
                        for k in range(KC):
                            nc.tensor.matmul(
                                ps[:],
                                w_sb[:, k, 128 * m : 128 * (m + 1)],
                                xbf[:, k],
                                start=(k == 0),
                                stop=(k == KC - 1),
                            )
                        t1 = small.tile([128, NPAIR], f32, name="t1", tag="t1")
                        nc.vector.tensor_mul(t1[:], ps[:], rstd_b[:])
                        t2 = small.tile([128, NPAIR], bf16, name="t2", tag="t2")
                        nc.vector.scalar_tensor_tensor(
                            t2[:], mr_b[:], ws_sb[:, m : m + 1], t1[:],
                            OP.mult, OP.add,
                        )
                        nc.scalar.activation(
                            dst[:, m, :], t2[:], AF.Identity,
                            bias=bias_sb[:, m : m + 1], scale=1.0,
                        )

                # ---- xn = x*rstd + mr (bf16; only feeds the V projection) ----
                xn = acts.tile([128, KC, NPAIR], bf16, name="xn", tag="xn")
                xnv = xn.rearrange("p k (b n) -> p k b n", b=2)
                for k in range(KC):
                    t = small.tile([128, NPAIR], f32, name="t_ln", tag="t1")
                    nc.vector.tensor_mul(
                        t.rearrange("p (b n) -> p b n", b=2), xfv[:, k], rstd_bv
                    )
                    nc.vector.tensor_add(
                        xnv[:, k], t.rearrange("p (b n) -> p b n", b=2), mr_bv
                    )

                # ---- V projection (token-major, per batch, 65-strided heads) ----
                v_sb = []
                for j in range(2):
                    v_c = []
                    for ci, (mstart, mlen) in enumerate(MCHUNKS):
                        vt = acts.tile(
                            [128, HEADS, DH + 1], bf16, name=f"v_{j}_{ci}",
                            tag=f"v_{j}_{ci}",
                        )
                        for s in range(2):  # halves of c_out (6 heads each)
                            pv = pp.tile(
                                [128, 384], f32, name="pv", tag="mm", bufs=2
                            )
                            for k in range(KC):
                                nc.tensor.matmul(
                                    pv[:mlen],
                                    xn[:, k, j * N + mstart : j * N + mstart + mlen],
                                    wv_sb[:, k, 384 * s : 384 * (s + 1)],
                                    start=(k == 0),
                                    stop=(k == KC - 1),
                                )
                            nc.scalar.activation(
                                vt[:mlen, 6 * s : 6 * (s + 1), 0:DH],
                                pv[:mlen].rearrange("p (h d) -> p h d", h=6),
                                AF.Identity, bias=zero_sb[:mlen], scale=1.0,
                            )
                        nc.vector.memset(vt[:mlen, :, DH : DH + 1], 1.0)
                        v_c.append(vt)
                    v_sb.append(v_c)

                # ---- attention: head-pairs packed into PE row groups;
                #      emission software-pipelined one pair ahead ----
                OT = acts.tile([128, KC, NPAIR], bf16, name="OT", tag="OT")

                def emit_sims(j, hp):
                    cb = j * N
                    sims, ets = [], []
                    for hh in range(2):  # heads 2*hp, 2*hp+1
                        prow = 64 * hh
                        sim = pp.tile(
                            [128, 2, N], f32, name="sim", tag=f"st{hh}"
                        )
                        for ci, (mstart, mlen) in enumerate(MCHUNKS):
                            nc.tensor.matmul(
                                sim[:mlen, ci],
                                kT[prow : prow + 64, hp,
                                   cb + mstart : cb + mstart + mlen],
                                qT[prow : prow + 64, hp, cb : cb + N],
                                start=True, stop=True,
                            )
                        et = small.tile(
                            [128, 2, N], bf16, name="et", tag=f"et{hh}"
                        )
                        nc.scalar.activation(
                            et[:], sim[:], AF.Exp, bias=0.0, scale=1.0
                        )
                        for ci, (mstart, mlen) in enumerate(MCHUNKS):
                            nc.vector.tensor_mul(
                                et[:mlen, ci], et[:mlen, ci],
                                eb_sb[ci][:mlen, 2 * hp + hh, :],
                            )
                        sims.append(sim)
                        ets.append(et)
                    return ets

                def emit_omms(j, hp, ets):
                    cb = j * N
                    for hh in range(2):
                        h = 2 * hp + hh
                        _otags = ["at0", "at1", "x0", "x1"]
                        ops = pp.tile(
                            [128, N], f32, name="ops", tag=_otags[(2 * hp + hh) % 4]
                        )
                        for ci, (mstart, mlen) in enumerate(MCHUNKS):
                            nc.tensor.matmul(
                                ops[: DH + 1],
                                v_sb[j][ci][:mlen, h, :],
                                ets[hh][:mlen, ci],
                                start=(ci == 0), stop=(ci == 1),
                            )
                        rec = small.tile([1, N], f32, name="rec", tag="rec")
                        nc.vector.reciprocal(rec[:], ops[DH : DH + 1, :])
                        rec_b = small.tile([64, N], f32, name="rec_b", tag="rec_b")
                        nc.gpsimd.partition_broadcast(rec_b[:], rec[:])
                        nc.vector.tensor_mul(
                            OT[64 * hh : 64 * hh + 64, hp, cb : cb + N],
                            ops[0:DH, :], rec_b[:],
                        )

                prev = None
                for j in range(2):
                    for hp in range(HEADS // 2):
                        ets = emit_sims(j, hp)
                        if prev is not None:
                            emit_omms(*prev)
                        prev = (j, hp, ets)
                emit_omms(*prev)

                # ---- out projection + residual 1 (bf16 residual) ----
                ybf = acts.tile([128, KC, NPAIR], bf16, name="ybf", tag="ybf")
                for m in range(KC):
                    po = pp.tile([128, NPAIR], f32, name="po", tag="mm", bufs=2)
                    for k in range(KC):
                        nc.tensor.matmul(
                            po[:],
                            wo_sb[:, k, 128 * m : 128 * (m + 1)],
                            OT[:, k, :],
                            start=(k == 0),
                            stop=(k == KC - 1),
                        )
                    nc.vector.scalar_tensor_tensor(
                        ybf[:, m, :].rearrange("p (b n) -> p b n", b=2),
                        po.rearrange("p (b n) -> p b n", b=2),
                        bo_sb[:, m : m + 1],
                        xfv[:, m],
                        OP.add, OP.add,
                    )

                # ---- FFN fused: h1 chunk -> gelu -> accumulate into 6 psum ----
                _ptags = ["st0", "st1", "at0", "at1", "x0", "x1"]
                pouts = [
                    pp.tile([128, NPAIR], f32, name=f"pout{o}", tag=_ptags[o])
                    for o in range(KC)
                ]
                for mf in range(KF):
                    p1 = pp.tile([128, NPAIR], f32, name="p1", tag="mm", bufs=2)
                    for k in range(KC):
                        nc.tensor.matmul(
                            p1[:],
                            w1_sb[:, k, 128 * mf : 128 * (mf + 1)],
                            ybf[:, k, :],
                            start=(k == 0),
                            stop=(k == KC - 1),
                        )
                    h1c = small.tile([128, NPAIR], bf16, name="h1c", tag="h1c")
                    nc.scalar.activation(
                        h1c[:], p1[:], AF.Gelu, bias=b1_sb[:, mf : mf + 1], scale=1.0
                    )
                    for o in range(KC):
                        nc.tensor.matmul(
                            pouts[o][:],
                            w2_sb[:, mf, 128 * o : 128 * (o + 1)],
                            h1c[:],
                            start=(mf == 0),
                            stop=(mf == KF - 1),
                        )

                # ---- residual 2 + per-chunk store ----
                o32 = xio.tile([128, 2 * KC, N], f32, name="o32", tag="o32", bufs=1)
                o32v = o32.rearrange("p (b k) n -> p k b n", b=2)
                for o in range(KC):
                    nc.vector.scalar_tensor_tensor(
                        o32v[:, o],
                        pouts[o].rearrange("p (b n) -> p b n", b=2),
                        b2_sb[:, o : o + 1],
                        ybf[:, o, :].rearrange("p (b n) -> p b n", b=2),
                        OP.add, OP.add,
                    )
                    nc.sync.dma_start(
                        out_d.ap()[b0 : b0 + 2, 128 * o : 128 * (o + 1)].rearrange(
                            "b (ko p) n -> p (b ko) n", p=128
                        ),
                        o32v[:, o],
                    )

    nc.finalize()
    return nc


_CACHE = {}


def prepare_in_maps(inputs):
    x = np.asarray(inputs["x"], dtype=np.float32)  # (64, 768, 14, 14)
    ln_g = np.asarray(inputs["ln_g"], dtype=np.float32)
    ln_b = np.asarray(inputs["ln_b"], dtype=np.float32)
    wq = np.asarray(inputs["wq"], dtype=np.float32)
    bq = np.asarray(inputs["bq"], dtype=np.float32)
    wk = np.asarray(inputs["wk"], dtype=np.float32)
    bk = np.asarray(inputs["bk"], dtype=np.float32)
    wv = np.asarray(inputs["wv"], dtype=np.float32)
    bv = np.asarray(inputs["bv"], dtype=np.float32)
    wo = np.asarray(inputs["wo"], dtype=np.float32)
    bo = np.asarray(inputs["bo"], dtype=np.float32)
    w1 = np.asarray(inputs["w1"], dtype=np.float32)
    b1 = np.asarray(inputs["b1"], dtype=np.float32)
    w2 = np.asarray(inputs["w2"], dtype=np.float32)
    b2 = np.asarray(inputs["b2"], dtype=np.float32)
    rel_bias = np.asarray(inputs["rel_bias"], dtype=np.float32)

    bf = ml_dtypes.bfloat16

    # Fold LayerNorm gamma into QKV weights, beta into their biases.
    wqp_f = ln_g[:, None] * wq
    wkp_f = ln_g[:, None] * wk
    wvp_f = ln_g[:, None] * wv
    bqp = (ln_b @ wq + bq).astype(np.float32)
    bkp = (ln_b @ wk + bk).astype(np.float32)
    bvp = (ln_b @ wv + bv).astype(np.float32)
    # V bias commutes through softmax (rows sum to 1): fold into out-proj bias.
    bop = (bo + bvp @ wo).astype(np.float32)
    # Column sums of the folded Q/K weights for the post-hoc mean correction.
    # Use the bf16-rounded weights so the correction matches the matmul.
    wqp = wqp_f.astype(bf)
    wkp = wkp_f.astype(bf)
    wsq = wqp.astype(np.float32).sum(axis=0).astype(np.float32)
    wsk = wkp.astype(np.float32).sum(axis=0).astype(np.float32)

    # Relative position bias, transposed per head, exponentiated.
    rel_idx = _relative_indices()
    bias = rel_bias[:, rel_idx]  # (HEADS, N, N) : bias[h, n, m]
    ebT = np.exp(bias.transpose(0, 2, 1))  # (HEADS, m, n)
    eb_m = ebT.transpose(1, 0, 2)  # (m, HEADS, n)
    eb0 = np.ascontiguousarray(eb_m[:M0]).astype(bf)
    eb1 = np.ascontiguousarray(eb_m[M0:]).astype(bf)

    common = {
        "wq": wqp, "wk": wkp, "wv": wvp_f.astype(bf),
        "wo": wo.astype(bf), "w1": w1.astype(bf), "w2": w2.astype(bf),
        "bq": bqp, "bk": bkp, "bo": bop,
        "b1": b1.astype(np.float32), "b2": b2.astype(np.float32),
        "wsq": wsq, "wsk": wsk,
        "eb0": eb0, "eb1": eb1,
    }

    x_flat = x.reshape(B, C, N)
    in_maps = []
    for c in range(NCORES):
        m = dict(common)
        m["x"] = np.ascontiguousarray(x_flat[c * BPC : (c + 1) * BPC])
        in_maps.append(m)
    return in_maps


def kernel(**inputs):
    import sys

    if "/opt/trn_rl_repo" not in sys.path:
        sys.path.insert(0, "/opt/trn_rl_repo")
    from concourse.bass_utils import run_bass_kernel_spmd

    in_maps = prepare_in_maps(inputs)

    if "nc" not in _CACHE:
        _CACHE["nc"] = _build_bass()
    nc = _CACHE["nc"]

    res = run_bass_kernel_spmd(nc, in_maps, core_ids=list(range(NCORES)))
    _CACHE["last_res"] = res
    outs = [r["out"] for r in res.results]
    full = np.concatenate(outs, axis=0)  # (64, 768, 196)
    return full.reshape(B, C, H, W).astype(np.float32)

